# revision 6
# baseline (speedup 1.0000x reference)
"""PixelCNN forward on 8 TRN2 NeuronCores.

Pipeline per image: masked conv7x7 (1->16, pad 3) -> ReLU ->
masked conv3x3 (16->16, pad 1) -> ReLU -> conv1x1 (16->1) -> sigmoid.

Sharding: pure data parallel, 128 images per core. On-core layout packs
8 images x 16 channels onto the 128 SBUF partitions; conv taps become
free-dim window offsets over zero-padded per-image spatial slabs, and
each tap is a block-diagonal matmul accumulated in PSUM.

conv1 (in_ch=1): the x slab is replicated 8x across partition groups,
pre-shifted by (ky, dx) for ky in 0..3, dx in 0..1, so each matmul
consumes 8 taps at once (4 rounds of K=128 cover the 4x7 masked grid).
"""

import numpy as np

import concourse.bass as bass
import concourse.bacc as bacc
import concourse.tile as tile
import concourse.mybir as mybir
from concourse.bass_utils import run_bass_kernel_spmd

AF = mybir.ActivationFunctionType
F32 = mybir.dt.float32

N_CORES = 8
B_PER_CORE = 128
IPG = 8               # images per group (on partitions)
NGROUPS = B_PER_CORE // IPG
H = W = 64
XP = 70               # padded x row length (pad 3)
H1P = 66              # padded h1 row length (pad 1)
XR_LEN = 4704         # xrep slab free length (>= 4687)
H1_LEN = H1P * H1P    # 4356
H2_LEN = H * W        # 4096
NCHUNK = 8            # 8 output rows per chunk -> 512 px
ROWS = 8
ROUNDS1 = (0, 2, 4, 6)
TAPS2 = ((0, 0), (0, 1), (0, 2), (1, 0), (1, 1))

# matmul operand dtype: fp32r (TF32-like) is 4x faster than fp32 at N=512.
# The BIR verifier requires fp32r matmul operands to be *produced* as fp32r,
# so all matmul-feeding tensors (x, weights, h1, h2) are declared float32r
# end-to-end; slab zeroing is done by DMA from a host zeros tensor because
# DVE memset cannot emit fp32r.
F32R = mybir.dt.float32r


def _masks():
    m1 = np.zeros((16, 1, 7, 7), np.float32)
    m1[:, :, :3, :] = 1.0
    m1[:, :, 3, :3] = 1.0
    m2 = np.zeros((16, 16, 3, 3), np.float32)
    m2[:, :, 0, :] = 1.0
    m2[:, :, 1, :2] = 1.0
    return m1, m2


def _prep(w1, b1, w2, b2, w3, b3):
    m1, m2 = _masks()
    w1g = (np.asarray(w1, np.float32) * m1)[:, 0]          # [16, 7, 7]
    w2g = np.asarray(w2, np.float32) * m2                  # [16, 16, 3, 3]
    w3v = np.asarray(w3, np.float32)[0, :, 0, 0]           # [16]

    w1m = np.zeros((128, 4 * 128), np.float32)
    for jj, j in enumerate(ROUNDS1):
        for r in range(8):
            ky, dx = divmod(r, 2)
            kx = j + dx
            if kx > 6:
                continue
            vals = w1g[:, ky, kx]                          # [16] per oc
            for i in range(IPG):
                w1m[r * 8 + i, jj * 128 + i * 16:jj * 128 + i * 16 + 16] = vals

    w2m = np.zeros((128, 5 * 128), np.float32)
    for tt, (ky, kx) in enumerate(TAPS2):
        blk = w2g[:, :, ky, kx].T                          # [ic, oc]
        for i in range(IPG):
            w2m[i * 16:(i + 1) * 16, tt * 128 + i * 16:tt * 128 + (i + 1) * 16] = blk

    w3m = np.zeros((128, 8), np.float32)
    for i in range(IPG):
        w3m[i * 16:(i + 1) * 16, i] = w3v

    b1x = np.tile(np.asarray(b1, np.float32), IPG).reshape(128, 1)
    b2x = np.tile(np.asarray(b2, np.float32), IPG).reshape(128, 1)
    b3x = np.full((8, 1), float(np.asarray(b3, np.float32)[0]), np.float32)
    return w1m, w2m, w3m, b1x, b2x, b3x


def build_nc():
    nc = bacc.Bacc("TRN2", target_bir_lowering=False, debug=False)

    x_d = nc.declare_dram_parameter("x", [B_PER_CORE, H, W], F32R, isOutput=False)
    w1m_d = nc.declare_dram_parameter("w1m", [128, 512], F32R, isOutput=False)
    w2m_d = nc.declare_dram_parameter("w2m", [128, 640], F32R, isOutput=False)
    w3m_d = nc.declare_dram_parameter("w3m", [128, 8], F32R, isOutput=False)
    b1_d = nc.declare_dram_parameter("b1x", [128, 1], F32, isOutput=False)
    b2_d = nc.declare_dram_parameter("b2x", [128, 1], F32, isOutput=False)
    b3_d = nc.declare_dram_parameter("b3x", [8, 1], F32, isOutput=False)
    z_d = nc.declare_dram_parameter("z", [128, XR_LEN], F32R, isOutput=False)
    o_d = nc.declare_dram_parameter("o", [B_PER_CORE, H * W], F32, isOutput=True)

    xr = [nc.alloc_sbuf_tensor(f"xr{k}", [128, XR_LEN], F32R) for k in range(2)]
    h1 = [nc.alloc_sbuf_tensor(f"h1_{k}", [128, H1_LEN], F32R) for k in range(2)]
    h2 = [nc.alloc_sbuf_tensor(f"h2_{k}", [128, H2_LEN], F32R) for k in range(2)]
    ob = nc.alloc_sbuf_tensor("ob", [8, 2 * H2_LEN], F32)
    w1s = nc.alloc_sbuf_tensor("w1s", [128, 512], F32R)
    w2s = nc.alloc_sbuf_tensor("w2s", [128, 640], F32R)
    w3s = nc.alloc_sbuf_tensor("w3s", [128, 8], F32R)
    b1s = nc.alloc_sbuf_tensor("b1s", [128, 1], F32)
    b2s = nc.alloc_sbuf_tensor("b2s", [128, 1], F32)
    b3s = nc.alloc_sbuf_tensor("b3s", [8, 1], F32)

    ps1 = [nc.alloc_psum_tensor(f"ps1_{k}", [128, 512], F32) for k in range(2)]
    ps2 = [nc.alloc_psum_tensor(f"ps2_{k}", [128, 512], F32) for k in range(2)]
    ps3 = [nc.alloc_psum_tensor(f"ps3_{k}", [8, 512], F32) for k in range(2)]

    with tile.TileContext(nc) as tc:
        for t in xr:
            nc.sync.dma_start(t.ap()[:], z_d.ap()[:])
        for t in h1:
            nc.sync.dma_start(t.ap()[:], z_d.ap()[:, :H1_LEN])
        nc.sync.dma_start(w1s.ap()[:], w1m_d.ap()[:])
        nc.sync.dma_start(w2s.ap()[:], w2m_d.ap()[:])
        nc.sync.dma_start(w3s.ap()[:], w3m_d.ap()[:])
        nc.sync.dma_start(b1s.ap()[:], b1_d.ap()[:])
        nc.sync.dma_start(b2s.ap()[:], b2_d.ap()[:])
        nc.sync.dma_start(b3s.ap()[:], b3_d.ap()[:])

        for g in range(NGROUPS):
            par = g % 2
            xrt, h1t, h2t = xr[par], h1[par], h2[par]

            # load x for this group: 8 shifted replicas of 8 images
            for r in range(8):
                ky, dx = divmod(r, 2)
                dst_off = (3 * XP + 3) - (XP * ky + dx)
                dst = bass.AP(tensor=xrt, offset=r * 8 * XR_LEN + dst_off,
                              ap=[[XR_LEN, 8], [XP, H], [1, W]])
                src = bass.AP(tensor=x_d, offset=g * 8 * H * W,
                              ap=[[H * W, 8], [W, H], [1, W]])
                nc.sync.dma_start(dst, src)

            # conv1: 4 accumulated matmuls per 8-row chunk
            for c in range(NCHUNK):
                y0 = c * ROWS
                ps = ps1[c % 2]
                for jj, j in enumerate(ROUNDS1):
                    lhs = w1s.ap()[:, jj * 128:(jj + 1) * 128]
                    rhs = bass.AP(tensor=xrt, offset=y0 * XP + j,
                                  ap=[[XR_LEN, 128], [XP, ROWS], [1, W]])
                    nc.tensor.matmul(ps.ap()[:], lhs, rhs,
                                     start=(jj == 0), stop=(jj == len(ROUNDS1) - 1))
                pin = bass.AP(tensor=ps, offset=0, ap=[[512, 128], [W, ROWS], [1, W]])
                hout = bass.AP(tensor=h1t, offset=(y0 + 1) * H1P + 1,
                               ap=[[H1_LEN, 128], [H1P, ROWS], [1, W]])
                nc.scalar.activation(hout, pin, AF.Relu, bias=b1s.ap()[:])

            # conv2: 5 accumulated matmuls per chunk
            for c in range(NCHUNK):
                y0 = c * ROWS
                ps = ps2[c % 2]
                for tt, (ky, kx) in enumerate(TAPS2):
                    lhs = w2s.ap()[:, tt * 128:(tt + 1) * 128]
                    rhs = bass.AP(tensor=h1t, offset=y0 * H1P + ky * H1P + kx,
                                  ap=[[H1_LEN, 128], [H1P, ROWS], [1, W]])
                    nc.tensor.matmul(ps.ap()[:], lhs, rhs,
                                     start=(tt == 0), stop=(tt == len(TAPS2) - 1))
                hout = bass.AP(tensor=h2t, offset=c * 512,
                               ap=[[H2_LEN, 128], [W, ROWS], [1, W]])
                pin = bass.AP(tensor=ps, offset=0, ap=[[512, 128], [W, ROWS], [1, W]])
                nc.scalar.activation(hout, pin, AF.Relu, bias=b2s.ap()[:])

            # conv3 + sigmoid
            for c in range(NCHUNK):
                ps = ps3[c % 2]
                rhs = h2t.ap()[:, c * 512:(c + 1) * 512]
                nc.tensor.matmul(ps.ap()[:], w3s.ap()[:], rhs,
                                 start=True, stop=True)
                oout = bass.AP(tensor=ob, offset=par * H2_LEN + c * 512,
                               ap=[[2 * H2_LEN, 8], [1, 512]])
                nc.scalar.activation(oout, ps.ap()[:], AF.Sigmoid, bias=b3s.ap()[:])

            src = bass.AP(tensor=ob, offset=par * H2_LEN,
                          ap=[[2 * H2_LEN, 8], [1, H2_LEN]])
            dst = bass.AP(tensor=o_d, offset=g * 8 * H * W,
                          ap=[[H * W, 8], [1, H * W]])
            nc.sync.dma_start(dst, src)

    nc.compile()
    return nc


_NC = None


def _get_nc():
    global _NC
    if _NC is None:
        _NC = build_nc()
    return _NC


def make_in_maps(x, w1, b1, w2, b2, w3, b3):
    x = np.asarray(x, np.float32)
    w1m, w2m, w3m, b1x, b2x, b3x = _prep(w1, b1, w2, b2, w3, b3)
    z = np.zeros((128, XR_LEN), np.float32)
    maps = []
    for c in range(N_CORES):
        xs = np.ascontiguousarray(
            x[c * B_PER_CORE:(c + 1) * B_PER_CORE, 0])
        maps.append({"x": xs, "w1m": w1m, "w2m": w2m, "w3m": w3m,
                     "b1x": b1x, "b2x": b2x, "b3x": b3x, "z": z})
    return maps


def kernel(x, w1, b1, w2, b2, w3, b3):
    nc = _get_nc()
    maps = make_in_maps(x, w1, b1, w2, b2, w3, b3)
    res = run_bass_kernel_spmd(nc, maps, core_ids=list(range(N_CORES)))
    out = np.concatenate(
        [res.results[c]["o"].reshape(B_PER_CORE, 1, H, W) for c in range(N_CORES)],
        axis=0)
    return np.ascontiguousarray(out.astype(np.float32))


# revision 22
# speedup vs baseline: 1.5943x; 1.5943x over previous
"""PixelCNN forward on 8 TRN2 NeuronCores.

Pipeline per image: masked conv7x7 (1->16, pad 3) -> ReLU ->
masked conv3x3 (16->16, pad 1) -> ReLU -> conv1x1 (16->1) -> sigmoid.

Sharding: pure data parallel, 128 images per core. On-core layout packs
8 images x 16 channels onto the 128 SBUF partitions; conv taps become
free-dim window offsets over zero-padded per-image spatial slabs, and
each tap is a block-diagonal matmul accumulated in PSUM.

conv1 (in_ch=1): the x slab is replicated 8x across partition groups,
pre-shifted by (ky, dx) for ky in 0..3, dx in 0..1, so each matmul
consumes 8 taps at once (4 rounds of K=128 cover the 4x7 masked grid).

v2: x is zero-padded host-side into xp [128, 4915] so each replica is a
single contiguous-row DMA (no slab zeroing, no partial writes); h1 pad
borders are zeroed once via ACT copies from a memset F32 buffer (legal
fp32r producer); the output DMA rides the Activation HWDGE queue so the
SP queue only carries input loads.
"""

import numpy as np

import concourse.bass as bass
import concourse.bacc as bacc
import concourse.tile as tile
import concourse.mybir as mybir
from concourse.bass_utils import run_bass_kernel_spmd

AF = mybir.ActivationFunctionType
F32 = mybir.dt.float32

N_CORES = 8
B_PER_CORE = 128
IPG = 8               # images per group (on partitions)
NGROUPS = B_PER_CORE // IPG
H = W = 64
XP = 70               # padded x row length (pad 3)
H1P = 66              # padded h1 row length (pad 1)
XR_LEN = 4704         # xrep slab free length (>= 4687)
XP_LEN = 4915         # host-padded x row length (max shift 211 + 4704)
H1_LEN = H1P * H1P    # 4356
H2_LEN = H * W        # 4096
NCHUNK = 8            # 8 output rows per chunk -> 512 px
ROWS = 8
ROUNDS1 = (0, 2, 4, 6)
TAPS2 = ((0, 0), (0, 1), (0, 2), (1, 0), (1, 1))

# matmul operand dtype: fp32r (TF32-like) is 4x faster than fp32 at N=512.
# The BIR verifier requires fp32r matmul operands to be *produced* as fp32r,
# so all matmul-feeding tensors (x, weights, h1, h2) are declared float32r
# end-to-end; slab zeroing is done by DMA from a host zeros tensor because
# DVE memset cannot emit fp32r.
F32R = mybir.dt.float32r


def _masks():
    m1 = np.zeros((16, 1, 7, 7), np.float32)
    m1[:, :, :3, :] = 1.0
    m1[:, :, 3, :3] = 1.0
    m2 = np.zeros((16, 16, 3, 3), np.float32)
    m2[:, :, 0, :] = 1.0
    m2[:, :, 1, :2] = 1.0
    return m1, m2


def _prep(w1, b1, w2, b2, w3, b3):
    m1, m2 = _masks()
    w1g = (np.asarray(w1, np.float32) * m1)[:, 0]          # [16, 7, 7]
    w2g = np.asarray(w2, np.float32) * m2                  # [16, 16, 3, 3]
    w3v = np.asarray(w3, np.float32)[0, :, 0, 0]           # [16]

    w1m = np.zeros((64, 4 * 128), np.float32)
    for jj, j in enumerate(ROUNDS1):
        for r in range(8):
            ky, dx = divmod(r, 2)
            kx = j + dx
            if kx > 6:
                continue
            vals = w1g[:, ky, kx]                          # [16] per oc
            for i in range(IPG):
                w1m[r * 8 + i, jj * 128 + i * 16:jj * 128 + i * 16 + 16] = vals

    w2m = np.zeros((128, 5 * 128), np.float32)
    for tt, (ky, kx) in enumerate(TAPS2):
        blk = w2g[:, :, ky, kx].T                          # [ic, oc]
        for i in range(IPG):
            w2m[i * 16:(i + 1) * 16, tt * 128 + i * 16:tt * 128 + (i + 1) * 16] = blk

    w3m = np.zeros((128, 8), np.float32)
    for i in range(IPG):
        w3m[i * 16:(i + 1) * 16, i] = w3v

    b1x = np.tile(np.asarray(b1, np.float32), IPG).reshape(128, 1)
    b2x = np.tile(np.asarray(b2, np.float32), IPG).reshape(128, 1)
    b3x = np.full((8, 1), float(np.asarray(b3, np.float32)[0]), np.float32)
    return w1m, w2m, w3m, b1x, b2x, b3x


def build_nc(repeats=1):
    nc = bacc.Bacc("TRN2", target_bir_lowering=False, debug=False)

    xp_d = nc.declare_dram_parameter("xp", [B_PER_CORE, XP_LEN], F32R, isOutput=False)
    w1m_d = nc.declare_dram_parameter("w1m", [64, 512], F32R, isOutput=False)
    w2m_d = nc.declare_dram_parameter("w2m", [128, 640], F32R, isOutput=False)
    w3m_d = nc.declare_dram_parameter("w3m", [128, 8], F32R, isOutput=False)
    b1_d = nc.declare_dram_parameter("b1x", [128, 1], F32, isOutput=False)
    b2_d = nc.declare_dram_parameter("b2x", [128, 1], F32, isOutput=False)
    b3_d = nc.declare_dram_parameter("b3x", [8, 1], F32, isOutput=False)
    o_d = nc.declare_dram_parameter("o", [B_PER_CORE, H * W], F32, isOutput=True)

    xr = [nc.alloc_sbuf_tensor(f"xr{k}", [64, XR_LEN], F32R) for k in range(2)]
    h1 = [nc.alloc_sbuf_tensor(f"h1_{k}", [128, H1_LEN], F32R) for k in range(2)]
    h2 = [nc.alloc_sbuf_tensor(f"h2_{k}", [128, H2_LEN], F32R) for k in range(2)]
    ob = nc.alloc_sbuf_tensor("ob", [8, 2 * H2_LEN], F32)
    w1s = nc.alloc_sbuf_tensor("w1s", [64, 512], F32R)
    w2s = nc.alloc_sbuf_tensor("w2s", [128, 640], F32R)
    w3s = nc.alloc_sbuf_tensor("w3s", [128, 8], F32R)
    b1s = nc.alloc_sbuf_tensor("b1s", [128, 1], F32)
    b2s = nc.alloc_sbuf_tensor("b2s", [128, 1], F32)
    b3s = nc.alloc_sbuf_tensor("b3s", [8, 1], F32)
    zbuf = nc.alloc_sbuf_tensor("zbuf", [128, 128], F32)

    ps1 = [nc.alloc_psum_tensor(f"ps1_{k}", [128, 512], F32) for k in range(2)]
    ps2 = [nc.alloc_psum_tensor(f"ps2_{k}", [128, 512], F32) for k in range(2)]
    ps3 = [nc.alloc_psum_tensor(f"ps3_{k}", [8, 512], F32) for k in range(2)]

    with tile.TileContext(nc) as tc:
        nc.vector.memset(zbuf.ap()[:], 0.0)
        # zero only the h1 pad borders (row 0, cols 0/65, row 65); conv1
        # evacuation writes the whole interior every group.
        for t in h1:
            for off, ap_out, ap_in in (
                (0, [[H1_LEN, 128], [1, 67]], [[128, 128], [1, 67]]),
                (131, [[H1_LEN, 128], [H1P, 64], [1, 2]],
                 [[128, 128], [2, 64], [1, 2]]),
                (4290, [[H1_LEN, 128], [1, 66]], [[128, 128], [1, 66]]),
            ):
                nc.scalar.activation(
                    bass.AP(tensor=t, offset=off, ap=ap_out),
                    bass.AP(tensor=zbuf, offset=0, ap=ap_in),
                    AF.Copy)
        nc.sync.dma_start(w1s.ap()[:], w1m_d.ap()[:])
        nc.sync.dma_start(w2s.ap()[:], w2m_d.ap()[:])
        nc.sync.dma_start(w3s.ap()[:], w3m_d.ap()[:])
        nc.sync.dma_start(b1s.ap()[:], b1_d.ap()[:])
        nc.sync.dma_start(b2s.ap()[:], b2_d.ap()[:])
        nc.sync.dma_start(b3s.ap()[:], b3_d.ap()[:])

        for g in range(NGROUPS * repeats):
            g = g % NGROUPS
            par = g % 2
            xrt, h1t, h2t = xr[par], h1[par], h2[par]

            # load x for this group: 8 shifted replicas of 8 images, each a
            # contiguous window of the host-padded row
            for r in range(8):
                ky, dx = divmod(r, 2)
                dst = bass.AP(tensor=xrt, offset=r * 8 * XR_LEN,
                              ap=[[XR_LEN, 8], [1, XR_LEN]])
                src = bass.AP(tensor=xp_d, offset=g * 8 * XP_LEN + XP * ky + dx,
                              ap=[[XP_LEN, 8], [1, XR_LEN]])
                nc.sync.dma_start(dst, src)

            # conv1: 4 accumulated matmuls per 8-row chunk
            for c in range(NCHUNK):
                y0 = c * ROWS
                ps = ps1[c % 2]
                for jj, j in enumerate(ROUNDS1):
                    lhs = w1s.ap()[:, jj * 128:(jj + 1) * 128]
                    rhs = bass.AP(tensor=xrt, offset=y0 * XP + j,
                                  ap=[[XR_LEN, 64], [XP, ROWS], [1, W]])
                    nc.tensor.matmul(ps.ap()[:], lhs, rhs,
                                     start=(jj == 0), stop=(jj == len(ROUNDS1) - 1))
                pin = bass.AP(tensor=ps, offset=0, ap=[[512, 128], [W, ROWS], [1, W]])
                hout = bass.AP(tensor=h1t, offset=(y0 + 1) * H1P + 1,
                               ap=[[H1_LEN, 128], [H1P, ROWS], [1, W]])
                nc.scalar.activation(hout, pin, AF.Relu, bias=b1s.ap()[:])

            # conv2: 5 accumulated matmuls per chunk
            for c in range(NCHUNK):
                y0 = c * ROWS
                ps = ps2[c % 2]
                for tt, (ky, kx) in enumerate(TAPS2):
                    lhs = w2s.ap()[:, tt * 128:(tt + 1) * 128]
                    rhs = bass.AP(tensor=h1t, offset=y0 * H1P + ky * H1P + kx,
                                  ap=[[H1_LEN, 128], [H1P, ROWS], [1, W]])
                    nc.tensor.matmul(ps.ap()[:], lhs, rhs,
                                     start=(tt == 0), stop=(tt == len(TAPS2) - 1))
                hout = bass.AP(tensor=h2t, offset=c * 512,
                               ap=[[H2_LEN, 128], [W, ROWS], [1, W]])
                pin = bass.AP(tensor=ps, offset=0, ap=[[512, 128], [W, ROWS], [1, W]])
                nc.scalar.activation(hout, pin, AF.Relu, bias=b2s.ap()[:])

            # conv3 + sigmoid
            for c in range(NCHUNK):
                ps = ps3[c % 2]
                rhs = h2t.ap()[:, c * 512:(c + 1) * 512]
                nc.tensor.matmul(ps.ap()[:], w3s.ap()[:], rhs,
                                 start=True, stop=True)
                oout = bass.AP(tensor=ob, offset=par * H2_LEN + c * 512,
                               ap=[[2 * H2_LEN, 8], [1, 512]])
                nc.scalar.activation(oout, ps.ap()[:], AF.Sigmoid, bias=b3s.ap()[:])

            src = bass.AP(tensor=ob, offset=par * H2_LEN,
                          ap=[[2 * H2_LEN, 8], [1, H2_LEN]])
            dst = bass.AP(tensor=o_d, offset=g * 8 * H * W,
                          ap=[[H * W, 8], [1, H * W]])
            nc.scalar.dma_start(dst, src)

    nc.compile()
    return nc


_NC = None


def _get_nc():
    global _NC
    if _NC is None:
        _NC = build_nc()
    return _NC


def make_in_maps(x, w1, b1, w2, b2, w3, b3):
    x = np.asarray(x, np.float32)
    w1m, w2m, w3m, b1x, b2x, b3x = _prep(w1, b1, w2, b2, w3, b3)
    maps = []
    for c in range(N_CORES):
        xs = x[c * B_PER_CORE:(c + 1) * B_PER_CORE, 0]
        P = np.zeros((B_PER_CORE, XP_LEN), np.float32)
        Pv = P[:, 3 * XP + 3:3 * XP + 3 + H * XP].reshape(B_PER_CORE, H, XP)
        Pv[:, :, :W] = xs
        maps.append({"xp": P, "w1m": w1m, "w2m": w2m, "w3m": w3m,
                     "b1x": b1x, "b2x": b2x, "b3x": b3x})
    return maps


def kernel(x, w1, b1, w2, b2, w3, b3):
    nc = _get_nc()
    maps = make_in_maps(x, w1, b1, w2, b2, w3, b3)
    res = run_bass_kernel_spmd(nc, maps, core_ids=list(range(N_CORES)))
    out = np.concatenate(
        [res.results[c]["o"].reshape(B_PER_CORE, 1, H, W) for c in range(N_CORES)],
        axis=0)
    return np.ascontiguousarray(out.astype(np.float32))
